# revision 19
# baseline (speedup 1.0000x reference)
"""Causal scaled-dot-product attention for Trainium2 (Bass/Tile), 8-core SPMD.

Problem: B=2, H=16, S=2048, D=128 fp32, causal mask, softmax(QK^T/sqrt(D)) @ V.
Sharding: batch*heads (32) split across 8 cores, 4 heads per core. Attention is
independent per (b,h): no communication.

Per-head algorithm (S^T layout -- avoids any transpose of the probability
matrix):
  - Q,K loaded via casting SWDGE DMA (f32->bf16) in a (p t) d layout, then
    transposed to Q^T,K^T [d=128 partitions, seq free] by a single XBAR
    DMA-transpose each -- no PE transposes, no DVE casts/copies.
  - V loaded via casting SWDGE DMA straight into its natural [k, d] tiles.
  - for each 512-wide query chunk c:
      for each key tile j (128 keys) at or below the diagonal:
        S^T[j] = K_j @ Q_c^T          (bf16 matmul, fp32 PSUM)
        P^T[j] = exp(S^T[j] / temp)   (ACT, PSUM->SBUF, bf16)
        diagonal tiles masked with an upper-triangular constant
        OUT^T  += V_j^T @ P^T[j]      (bf16 matmul, V in natural layout)
        den    += ones^T @ P^T[j]     (bf16 matmul, [1, 512])
      tail: rcq = 1/den; OUT^T * rcq (partition-broadcast multiply);
            4 bf16 PE transposes -> [q, d]; cast-copy to f32 -> DRAM
Softmax max-subtraction is skipped: logits are bounded (~20) so exp is safe in
fp32, and softmax is shift-invariant.

The XBAR trick: loading with s decomposed as (p t) -- p outer over partitions,
t inner -- makes the XBAR's reshape(reversed).T semantics produce a clean
[d, s] transpose in one instruction per tensor.

Emission is software-pipelined so the in-order PE never waits: PV/den matmuls
for group g are emitted after group g+1's QK/exp; chunk tails are deferred;
the next head's load + transposes are interleaved into the current head's main
loop. A burst of dummy matmuls at kernel start ramps the PE clock (HAM) out of
its cold 1.2 GHz state while the first DMAs land.
"""
from collections import deque

import numpy as np

import concourse.bacc as bacc
import concourse.tile as tile
import concourse.mybir as mybir
from concourse.bass_utils import run_bass_kernel_spmd
from concourse.masks import make_identity, make_upper_triangular

F32 = mybir.dt.float32
BF16 = mybir.dt.bfloat16
EXP = mybir.ActivationFunctionType.Exp

B, H, S, D = 2, 16, 2048, 128
TEMPERATURE = 11.313708498984761  # sqrt(128)
N_CORES = 8
HEADS_PER_CORE = (B * H) // N_CORES  # 4
P = 128                    # partitions / tile edge
CHUNK = 512                # query chunk (1 PSUM bank of fp32)
N_KT = S // P              # 16 key tiles per head
N_CH = S // CHUNK          # 4 query chunks per head
N_WARMUP = 26              # dummy matmuls to ramp the PE clock at start


def build_attention_nc(rep=1):
    nc = bacc.Bacc("TRN2", target_bir_lowering=False, debug=False,
                   num_devices=N_CORES)
    q_d = nc.dram_tensor("q", [HEADS_PER_CORE, S, D], F32, kind="ExternalInput").ap()
    k_d = nc.dram_tensor("k", [HEADS_PER_CORE, S, D], F32, kind="ExternalInput").ap()
    v_d = nc.dram_tensor("v", [HEADS_PER_CORE, S, D], F32, kind="ExternalInput").ap()
    o_d = nc.dram_tensor("out", [HEADS_PER_CORE, S, D], F32, kind="ExternalOutput").ap()

    n_heads = rep * HEADS_PER_CORE

    with tile.TileContext(nc) as tc:
        with tc.tile_pool(name="consts", bufs=1) as consts, \
             tc.tile_pool(name="inb", bufs=2) as inb, \
             tc.tile_pool(name="qkt", bufs=2) as qkt, \
             tc.tile_pool(name="px", bufs=6) as px, \
             tc.tile_pool(name="sm", bufs=4) as sm, \
             tc.tile_pool(name="ps_s", bufs=2, space="PSUM") as ps_s, \
             tc.tile_pool(name="ps_o", bufs=2, space="PSUM") as ps_o, \
             tc.tile_pool(name="ps_d", bufs=1, space="PSUM") as ps_d, \
             tc.tile_pool(name="ps_t", bufs=1, space="PSUM") as ps_t:

            # ---- constants ----
            utm = consts.tile([P, P], BF16)  # utm[k,q] = 1 iff q >= k
            make_upper_triangular(nc, utm, val=1.0, diag=True)
            ident_h = consts.tile([P, P], BF16)
            make_identity(nc, ident_h)
            ones_col = consts.tile([P, 1], BF16)
            nc.vector.memset(ones_col, 1.0)

            # ---- PE clock warmup: dummy bf16 matmuls while DMAs land ----
            for w in range(N_WARMUP):
                pw = ps_o.tile([P, CHUNK], F32, tag="po", name="po")
                nc.tensor.matmul(pw[:, 0:P], utm, utm, start=True, stop=True,
                                 skip_group_check=True)



            head_state = {}

            def emit_load(hh, first=False):
                h = hh % HEADS_PER_CORE
                qT = qkt.tile([P, S], BF16, tag="qT", name="qT")
                kT = qkt.tile([P, S], BF16, tag="kT", name="kT")
                vnr = qkt.tile([P, N_KT, P], BF16, tag="vnr", name="vnr")
                qsd = inb.tile([P, N_KT, P], BF16, tag="qsd", name="qsd")
                ksd = inb.tile([P, N_KT, P], BF16, tag="ksd", name="ksd")
                vn = inb.tile([P, N_KT, P], F32, tag="vn", name="vn")
                st = dict(qsd=qsd, ksd=ksd, vn=vn, qT=qT, kT=kT, vnr=vnr)
                head_state[hh] = st
                # V rides the parallel HWDGE ring (cast later on DVE) so the
                # serial SWDGE stream only carries Q and K
                nc.sync.dma_start(
                    out=vn, in_=v_d[h].rearrange("(t p) d -> p t d", p=P))
                # SWDGE is serial: interleave split casting loads so kT/qT
                # group g lands just before its transpose is popped
                nq = 4 if first else 2
                for part in range(nq):
                    lo = part * (S // nq)
                    hp = part * (N_KT // nq)
                    for dst, srcd in ((ksd, k_d), (qsd, q_d)):
                        nc.gpsimd.dma_start(
                            out=dst[:, hp:hp + N_KT // nq],
                            in_=srcd[h, lo:lo + S // nq].rearrange(
                                "(t p) d -> p t d", p=P))

            def transpose_task(hh, src_key, dst_key, g):
                def t():
                    st = head_state[hh]
                    srcT, dst = st[src_key], st[dst_key]
                    ptr = ps_t.tile([P, CHUNK], BF16, tag="ptr", name="ptr")
                    for t4 in range(4):
                        tt = 4 * g + t4
                        nc.tensor.transpose(
                            ptr[:, t4 * P:(t4 + 1) * P],
                            srcT[:, tt, :], ident_h)
                    nc.vector.tensor_copy(
                        dst[:, g * CHUNK:(g + 1) * CHUNK], ptr)
                return t

            def vcast_task(hh, lo, hi):
                def tv():
                    st = head_state[hh]
                    nc.vector.tensor_copy(st["vnr"][:, lo:hi],
                                          st["vn"][:, lo:hi])
                return tv

            def prep_tasks(hh):
                return [
                    transpose_task(hh, "ksd", "kT", 0),
                    transpose_task(hh, "qsd", "qT", 0),
                    vcast_task(hh, 0, N_KT // 2),
                    transpose_task(hh, "ksd", "kT", 1),
                    transpose_task(hh, "qsd", "qT", 1),
                    transpose_task(hh, "ksd", "kT", 2),
                    transpose_task(hh, "qsd", "qT", 2),
                    vcast_task(hh, N_KT // 2, N_KT),
                    transpose_task(hh, "ksd", "kT", 3),
                    transpose_task(hh, "qsd", "qT", 3),
                ]

            def make_pv(st, offs, pexp, psum_o, psum_d, jmax):
                def emit():
                    for (j, oj, base) in offs:
                        nc.tensor.matmul(
                            psum_o[:, oj:CHUNK], st["vnr"][:, j, :],
                            pexp[:, base + oj:base + CHUNK],
                            start=(j == 0), stop=(j == jmax),
                            skip_group_check=True)
                        nc.tensor.matmul(
                            psum_d[:, oj:CHUNK], ones_col,
                            pexp[:, base + oj:base + CHUNK],
                            start=(j == 0), stop=(j == jmax),
                            skip_group_check=True)
                return emit

            def make_tail(hh, c, psum_o, psum_d):
                def emit():
                    h = hh % HEADS_PER_CORE
                    # denominators out of PSUM first (releases the den bank),
                    # then reciprocal once per chunk
                    dens = sm.tile([1, CHUNK], F32, tag="dens", name="dens")
                    nc.vector.tensor_copy(dens, psum_d)
                    rcq = sm.tile([1, CHUNK], F32, tag="rcq", name="rcq")
                    nc.vector.reciprocal_approx_fast(rcq, dens)
                    # physically replicate 1/den across partitions (gpsimd),
                    # then normalize OUT^T columns with one elementwise mul
                    rcb = sm.tile([P, CHUNK], F32, tag="rcb", name="rcb")
                    nc.gpsimd.partition_broadcast(rcb, rcq)
                    outn = sm.tile([P, CHUNK], BF16, tag="outn", name="outn")
                    nc.vector.tensor_mul(outn, psum_o, rcb)
                    # transpose OUT^T back to [q, d] (bf16, cheap)
                    ptr2 = ps_t.tile([P, CHUNK], BF16, tag="ptr", name="ptr")
                    for tt in range(4):
                        nc.tensor.transpose(
                            ptr2[:, tt * P:(tt + 1) * P],
                            outn[:, tt * P:(tt + 1) * P], ident_h)
                    # evacuate + widen to f32 for the store
                    outT = sm.tile([P, 4, P], F32, tag="outT", name="outT")
                    nc.vector.tensor_copy(
                        outT, ptr2.rearrange("p (a b) -> p a b", b=P))
                    nc.sync.dma_start(
                        out=o_d[h, CHUNK * c:CHUNK * (c + 1), :].rearrange(
                            "(t p) d -> p t d", p=P),
                        in_=outT)
                return emit

            # head 0: issue split loads; first k/q transpose groups
            # upfront, rest popped during the main loop.
            emit_load(0, first=True)
            head0_tasks = prep_tasks(0)
            for t in head0_tasks[:3]:
                t()
            pending_prep = deque(head0_tasks[3:])

            for hh in range(n_heads):
                st = head_state[hh]
                if hh + 1 < n_heads:
                    emit_load(hh + 1)
                    pending_prep.extend(prep_tasks(hh + 1))

                pending_pv = None          # PV/den of previous group
                deferred = []              # [(age, closure)] chunk tails
                group_idx = 0

                for c in range(N_CH):
                    jmax = 4 * c + 3
                    psum_o = ps_o.tile([P, CHUNK], F32, tag="po", name="po")
                    psum_d = ps_d.tile([1, CHUNK], F32, tag="pd", name="pd")

                    for jp in range((jmax + 2) // 2):
                        j0 = 2 * jp
                        js = [j for j in (j0, j0 + 1) if j <= jmax]
                        psum_s = ps_s.tile([P, 2 * CHUNK], F32, tag="psm",
                                           name="psm")
                        pexp = px.tile([P, 2 * CHUNK], BF16, tag="pexp",
                                       name="pexp")

                        offs = []
                        for j in js:
                            oj = max(0, P * j - CHUNK * c)
                            base = (j - j0) * CHUNK
                            offs.append((j, oj, base))
                            # full-width write even on diagonal tiles: the
                            # below-diagonal columns hold bounded garbage that
                            # exp processes but PV/den never consume; this
                            # keeps every exp read covered by this tile's
                            # writes (no stale-PSUM reads)
                            nc.tensor.matmul(
                                psum_s[:, base:base + CHUNK],
                                st["kT"][:, j * P:(j + 1) * P],
                                st["qT"][:, CHUNK * c:CHUNK * (c + 1)],
                                start=True, stop=True)

                        # exp (+ causal masking of diagonal 128-blocks,
                        # applied in place after the exp). Diagonal pairs are
                        # exp'd in a single wide op spanning both j regions;
                        # the dead gap between them is never consumed.
                        diag = any(j * P >= CHUNK * c for (j, oj, base) in offs)
                        lo = offs[0][2] + offs[0][1]
                        hi = offs[-1][2] + CHUNK
                        nc.scalar.activation(
                            pexp[:, lo:hi], psum_s[:, lo:hi],
                            EXP, scale=1.0 / TEMPERATURE)
                        if diag:
                            for (j, oj, base) in offs:
                                if j * P >= CHUNK * c:
                                    nc.vector.tensor_mul(
                                        pexp[:, base + oj:base + oj + P],
                                        pexp[:, base + oj:base + oj + P], utm)

                        if pending_pv is not None:
                            pending_pv()
                        pending_pv = make_pv(st, offs, pexp, psum_o, psum_d,
                                             jmax)

                        group_idx += 1
                        if pending_prep and (hh > 0 or group_idx % 2 == 0
                                             or group_idx <= 2):
                            pending_prep.popleft()()
                        tail_age = 1 if hh == n_heads - 1 else 2
                        for item in list(deferred):
                            if group_idx - item[0] >= tail_age:
                                item[1]()
                                deferred.remove(item)

                    deferred.append((group_idx, make_tail(hh, c, psum_o,
                                                          psum_d)))

                # flush this head
                if pending_pv is not None:
                    pending_pv()
                for item in deferred:
                    item[1]()
            while pending_prep:
                pending_prep.popleft()()

    nc.compile()
    return nc


_NC_CACHE = None


def _get_nc():
    global _NC_CACHE
    if _NC_CACHE is None:
        _NC_CACHE = build_attention_nc()
    return _NC_CACHE


def kernel(q, k, v, mask=None, _trace=False):
    """Full-input entry point: q,k,v [2,16,2048,128] f32, mask [2,1,2048,2048]
    int32 (causal; the kernel hardcodes causality and does not read it).
    Returns [2,16,2048,128] f32."""
    nc = _get_nc()
    qf = np.ascontiguousarray(np.asarray(q, dtype=np.float32).reshape(B * H, S, D))
    kf = np.ascontiguousarray(np.asarray(k, dtype=np.float32).reshape(B * H, S, D))
    vf = np.ascontiguousarray(np.asarray(v, dtype=np.float32).reshape(B * H, S, D))
    in_maps = []
    for i in range(N_CORES):
        sl = slice(i * HEADS_PER_CORE, (i + 1) * HEADS_PER_CORE)
        in_maps.append({"q": qf[sl], "k": kf[sl], "v": vf[sl]})
    res = run_bass_kernel_spmd(nc, in_maps, list(range(N_CORES)), trace=_trace)
    out = np.concatenate([res.results[i]["out"] for i in range(N_CORES)], axis=0)
    out = out.reshape(B, H, S, D).astype(np.float32)
    if _trace:
        return out, res
    return out


# revision 20
# speedup vs baseline: 1.0172x; 1.0172x over previous
"""Causal scaled-dot-product attention for Trainium2 (Bass/Tile), 8-core SPMD.

Problem: B=2, H=16, S=2048, D=128 fp32, causal mask, softmax(QK^T/sqrt(D)) @ V.
Sharding: batch*heads (32) split across 8 cores, 4 heads per core. Attention is
independent per (b,h): no communication.

Per-head algorithm (S^T layout -- avoids any transpose of the probability
matrix):
  - Q,K loaded via casting SWDGE DMA (f32->bf16) in a (p t) d layout, then
    transposed to Q^T,K^T [d=128 partitions, seq free] by a single XBAR
    DMA-transpose each -- no PE transposes, no DVE casts/copies.
  - V loaded via casting SWDGE DMA straight into its natural [k, d] tiles.
  - for each 512-wide query chunk c:
      for each key tile j (128 keys) at or below the diagonal:
        S^T[j] = K_j @ Q_c^T          (bf16 matmul, fp32 PSUM)
        P^T[j] = exp(S^T[j] / temp)   (ACT, PSUM->SBUF, bf16)
        diagonal tiles masked with an upper-triangular constant
        OUT^T  += V_j^T @ P^T[j]      (bf16 matmul, V in natural layout)
        den    += ones^T @ P^T[j]     (bf16 matmul, [1, 512])
      tail: rcq = 1/den; OUT^T * rcq (partition-broadcast multiply);
            4 bf16 PE transposes -> [q, d]; cast-copy to f32 -> DRAM
Softmax max-subtraction is skipped: logits are bounded (~20) so exp is safe in
fp32, and softmax is shift-invariant.

The XBAR trick: loading with s decomposed as (p t) -- p outer over partitions,
t inner -- makes the XBAR's reshape(reversed).T semantics produce a clean
[d, s] transpose in one instruction per tensor.

Emission is software-pipelined so the in-order PE never waits: PV/den matmuls
for group g are emitted after group g+1's QK/exp; chunk tails are deferred;
the next head's load + transposes are interleaved into the current head's main
loop. A burst of dummy matmuls at kernel start ramps the PE clock (HAM) out of
its cold 1.2 GHz state while the first DMAs land.
"""
from collections import deque

import numpy as np

import concourse.bacc as bacc
import concourse.tile as tile
import concourse.mybir as mybir
from concourse.bass_utils import run_bass_kernel_spmd
from concourse.masks import make_identity, make_upper_triangular

F32 = mybir.dt.float32
BF16 = mybir.dt.bfloat16
EXP = mybir.ActivationFunctionType.Exp

B, H, S, D = 2, 16, 2048, 128
TEMPERATURE = 11.313708498984761  # sqrt(128)
N_CORES = 8
HEADS_PER_CORE = (B * H) // N_CORES  # 4
P = 128                    # partitions / tile edge
CHUNK = 512                # query chunk (1 PSUM bank of fp32)
N_KT = S // P              # 16 key tiles per head
N_CH = S // CHUNK          # 4 query chunks per head
N_WARMUP = 26              # dummy matmuls to ramp the PE clock at start


def build_attention_nc(rep=1):
    nc = bacc.Bacc("TRN2", target_bir_lowering=False, debug=False,
                   num_devices=N_CORES)
    q_d = nc.dram_tensor("q", [HEADS_PER_CORE, S, D], F32, kind="ExternalInput").ap()
    k_d = nc.dram_tensor("k", [HEADS_PER_CORE, S, D], F32, kind="ExternalInput").ap()
    v_d = nc.dram_tensor("v", [HEADS_PER_CORE, S, D], F32, kind="ExternalInput").ap()
    o_d = nc.dram_tensor("out", [HEADS_PER_CORE, S, D], F32, kind="ExternalOutput").ap()

    n_heads = rep * HEADS_PER_CORE

    with tile.TileContext(nc) as tc:
        with tc.tile_pool(name="consts", bufs=1) as consts, \
             tc.tile_pool(name="inb", bufs=2) as inb, \
             tc.tile_pool(name="qkt", bufs=2) as qkt, \
             tc.tile_pool(name="px", bufs=6) as px, \
             tc.tile_pool(name="sm", bufs=4) as sm, \
             tc.tile_pool(name="ps_s", bufs=2, space="PSUM") as ps_s, \
             tc.tile_pool(name="ps_o", bufs=2, space="PSUM") as ps_o, \
             tc.tile_pool(name="ps_d", bufs=1, space="PSUM") as ps_d, \
             tc.tile_pool(name="ps_t", bufs=1, space="PSUM") as ps_t:

            # ---- constants ----
            utm = consts.tile([P, P], BF16)  # utm[k,q] = 1 iff q >= k
            make_upper_triangular(nc, utm, val=1.0, diag=True)
            ident_h = consts.tile([P, P], BF16)
            make_identity(nc, ident_h)
            ones_col = consts.tile([P, 1], BF16)
            nc.vector.memset(ones_col, 1.0)

            # ---- PE clock warmup: dummy bf16 matmuls while DMAs land ----
            for w in range(N_WARMUP):
                pw = ps_o.tile([P, CHUNK], F32, tag="po", name="po")
                nc.tensor.matmul(pw[:, 0:P], utm, utm, start=True, stop=True,
                                 skip_group_check=True)



            head_state = {}

            def emit_load(hh, first=False):
                h = hh % HEADS_PER_CORE
                qT = qkt.tile([P, S], BF16, tag="qT", name="qT")
                kT = qkt.tile([P, S], BF16, tag="kT", name="kT")
                vnr = qkt.tile([P, N_KT, P], BF16, tag="vnr", name="vnr")
                qsd = inb.tile([P, N_KT, P], BF16, tag="qsd", name="qsd")
                ksd = inb.tile([P, N_KT, P], BF16, tag="ksd", name="ksd")
                st = dict(qsd=qsd, ksd=ksd, qT=qT, kT=kT, vnr=vnr)
                head_state[hh] = st
                if first:
                    # SWDGE is serial: interleave quarter-tensor casting
                    # loads so kT/qT group g and the first V tiles land early
                    for qtr in range(4):
                        lo = qtr * (S // 4)
                        h4 = qtr * (N_KT // 4)
                        for dst, srcd in ((ksd, k_d), (qsd, q_d), (vnr, v_d)):
                            nc.gpsimd.dma_start(
                                out=dst[:, h4:h4 + N_KT // 4],
                                in_=srcd[h, lo:lo + S // 4].rearrange(
                                    "(t p) d -> p t d", p=P))
                else:
                    for dst, srcd in ((ksd, k_d), (qsd, q_d), (vnr, v_d)):
                        nc.gpsimd.dma_start(
                            out=dst, in_=srcd[h].rearrange(
                                "(t p) d -> p t d", p=P))

            def transpose_task(hh, src_key, dst_key, g):
                def t():
                    st = head_state[hh]
                    srcT, dst = st[src_key], st[dst_key]
                    ptr = ps_t.tile([P, CHUNK], BF16, tag="ptr", name="ptr")
                    for t4 in range(4):
                        tt = 4 * g + t4
                        nc.tensor.transpose(
                            ptr[:, t4 * P:(t4 + 1) * P],
                            srcT[:, tt, :], ident_h)
                    nc.vector.tensor_copy(
                        dst[:, g * CHUNK:(g + 1) * CHUNK], ptr)
                return t

            def prep_tasks(hh):
                return [
                    transpose_task(hh, "ksd", "kT", 0),
                    transpose_task(hh, "qsd", "qT", 0),
                    transpose_task(hh, "ksd", "kT", 1),
                    transpose_task(hh, "qsd", "qT", 1),
                    transpose_task(hh, "ksd", "kT", 2),
                    transpose_task(hh, "qsd", "qT", 2),
                    transpose_task(hh, "ksd", "kT", 3),
                    transpose_task(hh, "qsd", "qT", 3),
                ]

            def make_pv(st, offs, pexp, psum_o, psum_d, jmax):
                def emit():
                    for (j, oj, base) in offs:
                        nc.tensor.matmul(
                            psum_o[:, oj:CHUNK], st["vnr"][:, j, :],
                            pexp[:, base + oj:base + CHUNK],
                            start=(j == 0), stop=(j == jmax),
                            skip_group_check=True)
                        nc.tensor.matmul(
                            psum_d[:, oj:CHUNK], ones_col,
                            pexp[:, base + oj:base + CHUNK],
                            start=(j == 0), stop=(j == jmax),
                            skip_group_check=True)
                return emit

            def make_tail(hh, c, psum_o, psum_d):
                def emit():
                    h = hh % HEADS_PER_CORE
                    # denominators out of PSUM first (releases the den bank),
                    # then reciprocal once per chunk
                    dens = sm.tile([1, CHUNK], F32, tag="dens", name="dens")
                    nc.vector.tensor_copy(dens, psum_d)
                    rcq = sm.tile([1, CHUNK], F32, tag="rcq", name="rcq")
                    nc.vector.reciprocal_approx_fast(rcq, dens)
                    # physically replicate 1/den across partitions (gpsimd),
                    # then normalize OUT^T columns with one elementwise mul
                    rcb = sm.tile([P, CHUNK], F32, tag="rcb", name="rcb")
                    nc.gpsimd.partition_broadcast(rcb, rcq)
                    outn = sm.tile([P, CHUNK], BF16, tag="outn", name="outn")
                    nc.vector.tensor_mul(outn, psum_o, rcb)
                    # transpose OUT^T back to [q, d] (bf16, cheap)
                    ptr2 = ps_t.tile([P, CHUNK], BF16, tag="ptr", name="ptr")
                    for tt in range(4):
                        nc.tensor.transpose(
                            ptr2[:, tt * P:(tt + 1) * P],
                            outn[:, tt * P:(tt + 1) * P], ident_h)
                    # evacuate + widen to f32 for the store
                    outT = sm.tile([P, 4, P], F32, tag="outT", name="outT")
                    nc.vector.tensor_copy(
                        outT, ptr2.rearrange("p (a b) -> p a b", b=P))
                    nc.sync.dma_start(
                        out=o_d[h, CHUNK * c:CHUNK * (c + 1), :].rearrange(
                            "(t p) d -> p t d", p=P),
                        in_=outT)
                return emit

            # head 0: issue split loads; first k/q transpose groups
            # upfront, rest popped during the main loop.
            emit_load(0, first=True)
            head0_tasks = prep_tasks(0)
            for t in head0_tasks[:2]:
                t()
            pending_prep = deque(head0_tasks[2:])

            for hh in range(n_heads):
                st = head_state[hh]
                if hh + 1 < n_heads:
                    emit_load(hh + 1)
                    pending_prep.extend(prep_tasks(hh + 1))

                pending_pv = None          # PV/den of previous group
                deferred = []              # [(age, closure)] chunk tails
                group_idx = 0

                for c in range(N_CH):
                    jmax = 4 * c + 3
                    psum_o = ps_o.tile([P, CHUNK], F32, tag="po", name="po")
                    psum_d = ps_d.tile([1, CHUNK], F32, tag="pd", name="pd")

                    for jp in range((jmax + 2) // 2):
                        j0 = 2 * jp
                        js = [j for j in (j0, j0 + 1) if j <= jmax]
                        psum_s = ps_s.tile([P, 2 * CHUNK], F32, tag="psm",
                                           name="psm")
                        pexp = px.tile([P, 2 * CHUNK], BF16, tag="pexp",
                                       name="pexp")

                        offs = []
                        for j in js:
                            oj = max(0, P * j - CHUNK * c)
                            base = (j - j0) * CHUNK
                            offs.append((j, oj, base))
                            # full-width write even on diagonal tiles: the
                            # below-diagonal columns hold bounded garbage that
                            # exp processes but PV/den never consume; this
                            # keeps every exp read covered by this tile's
                            # writes (no stale-PSUM reads)
                            nc.tensor.matmul(
                                psum_s[:, base:base + CHUNK],
                                st["kT"][:, j * P:(j + 1) * P],
                                st["qT"][:, CHUNK * c:CHUNK * (c + 1)],
                                start=True, stop=True)

                        # exp (+ causal masking of diagonal 128-blocks,
                        # applied in place after the exp). Diagonal pairs are
                        # exp'd in a single wide op spanning both j regions;
                        # the dead gap between them is never consumed.
                        diag = any(j * P >= CHUNK * c for (j, oj, base) in offs)
                        lo = offs[0][2] + offs[0][1]
                        hi = offs[-1][2] + CHUNK
                        nc.scalar.activation(
                            pexp[:, lo:hi], psum_s[:, lo:hi],
                            EXP, scale=1.0 / TEMPERATURE)
                        if diag:
                            for (j, oj, base) in offs:
                                if j * P >= CHUNK * c:
                                    nc.vector.tensor_mul(
                                        pexp[:, base + oj:base + oj + P],
                                        pexp[:, base + oj:base + oj + P], utm)

                        if pending_pv is not None:
                            pending_pv()
                        pending_pv = make_pv(st, offs, pexp, psum_o, psum_d,
                                             jmax)

                        group_idx += 1
                        if pending_prep and (hh > 0 or group_idx % 2 == 0
                                             or group_idx <= 2):
                            pending_prep.popleft()()
                        tail_age = 1 if hh == n_heads - 1 else 2
                        for item in list(deferred):
                            if group_idx - item[0] >= tail_age:
                                item[1]()
                                deferred.remove(item)

                    deferred.append((group_idx, make_tail(hh, c, psum_o,
                                                          psum_d)))

                # flush this head
                if pending_pv is not None:
                    pending_pv()
                for item in deferred:
                    item[1]()
            while pending_prep:
                pending_prep.popleft()()

    nc.compile()
    return nc


_NC_CACHE = None


def _get_nc():
    global _NC_CACHE
    if _NC_CACHE is None:
        _NC_CACHE = build_attention_nc()
    return _NC_CACHE


def kernel(q, k, v, mask=None, _trace=False):
    """Full-input entry point: q,k,v [2,16,2048,128] f32, mask [2,1,2048,2048]
    int32 (causal; the kernel hardcodes causality and does not read it).
    Returns [2,16,2048,128] f32."""
    nc = _get_nc()
    qf = np.ascontiguousarray(np.asarray(q, dtype=np.float32).reshape(B * H, S, D))
    kf = np.ascontiguousarray(np.asarray(k, dtype=np.float32).reshape(B * H, S, D))
    vf = np.ascontiguousarray(np.asarray(v, dtype=np.float32).reshape(B * H, S, D))
    in_maps = []
    for i in range(N_CORES):
        sl = slice(i * HEADS_PER_CORE, (i + 1) * HEADS_PER_CORE)
        in_maps.append({"q": qf[sl], "k": kf[sl], "v": vf[sl]})
    res = run_bass_kernel_spmd(nc, in_maps, list(range(N_CORES)), trace=_trace)
    out = np.concatenate([res.results[i]["out"] for i in range(N_CORES)], axis=0)
    out = out.reshape(B, H, S, D).astype(np.float32)
    if _trace:
        return out, res
    return out
